# revision 4
# baseline (speedup 1.0000x reference)
"""Trainium2 Bass kernel for a 2-layer LSTMCell autoencoder (batch=1).

Reference computation:
    h1, c1 = LSTMCell1(x, (h_t, c_t))      # input 4000 -> hidden 5000
    h2, c2 = LSTMCell2(h1, (h2_t, c2_t))   # hidden 5000 -> hidden 5000
    out = h2 @ w_lin.T + b_lin             # hidden 5000 -> 4000

Strategy (8 NeuronCores, tensor-parallel on the 4H gate dim):
  - Core r owns gate slice [r*625:(r+1)*625] of each of the i/f/g/o gates
    (2500 gate outputs per core per cell).
  - All matvecs run on the TensorEngine as psum[1,N] += vec[128,1].T @ W[128,N]
    with the weights streamed from HBM as the moving operand. Weights are
    pre-transposed/padded on the host so each SBUF tile DMA is contiguous.
  - Biases are folded in as an extra weight row against a 1.0 vec element.
  - h1 / h2 are all-gathered (640 elems/rank: 625 + 1.0 + padding) so each
    core has the full hidden vector for the next matvec.
  - The final linear is column-parallel: core r computes out[r*500:(r+1)*500]
    directly from the gathered h2; no output collective needed.

kernel(**inputs) takes the full unsharded inputs and returns the full output.
"""
import sys
import types

sys.path.insert(0, "/opt/trn_rl_repo")

import numpy as np

import concourse.bacc as bacc
import concourse.tile as tile
import concourse.mybir as mybir
from concourse.bass_utils import run_bass_kernel_spmd

N_CORES = 8
I_DIM = 4000
H_DIM = 5000
HS = H_DIM // N_CORES          # 625 per-core slice of each gate
C = 4 * HS                     # 2500 gate outputs per core per cell
OS = I_DIM // N_CORES          # 500 output slice per core
SEG = 640                      # padded per-rank AG segment (625 + 1 + 14)
GATH = SEG * N_CORES           # 5120 gathered (and 128-aligned) hidden vec

# vec1 = [x (4000), 1.0, pad -> 4096 | h_t (5000), pad -> 5120]
XSEG = 4096
HSEG = 5120
R1 = XSEG + HSEG               # 9216 rows of W1, 72 k-blocks
R2 = GATH + HSEG               # 10240 rows of W2, 80 k-blocks
RL = GATH                      # 5120 rows of W_lin, 40 k-blocks
B1 = R1 // 128                 # 72
B2 = R2 // 128                 # 80
BL = RL // 128                 # 40
NCHUNK = C // 500              # 5 matmul chunks of 500 per gate row-block
BPD = 6                        # k-blocks per weight DMA

DT = mybir.dt.float32
F32 = np.float32

_CACHED_NC = None


def _build_bass():
    """Build the (input-independent) SPMD Bass graph once."""
    nc = bacc.Bacc("TRN2", target_bir_lowering=False, debug=False,
                   num_devices=N_CORES)

    w1_ext = nc.dram_tensor("w1", [R1, C], DT, kind="ExternalInput")
    w2_ext = nc.dram_tensor("w2", [R2, C], DT, kind="ExternalInput")
    wl_ext = nc.dram_tensor("wl", [RL, OS], DT, kind="ExternalInput")
    vec1_ext = nc.dram_tensor("vec1", [128, B1], DT, kind="ExternalInput")
    h2t_ext = nc.dram_tensor("h2t", [128, BL], DT, kind="ExternalInput")
    c1s_ext = nc.dram_tensor("c1s", [1, HS], DT, kind="ExternalInput")
    c2s_ext = nc.dram_tensor("c2s", [1, HS], DT, kind="ExternalInput")
    out_ext = nc.dram_tensor("out", [1, OS], DT, kind="ExternalOutput")

    h1_bounce = nc.dram_tensor("h1_bounce", [SEG], DT)
    h1_gath = nc.dram_tensor("h1_gath", [GATH], DT, addr_space="Shared")
    h2_bounce = nc.dram_tensor("h2_bounce", [SEG], DT)
    h2_gath = nc.dram_tensor("h2_gath", [GATH], DT, addr_space="Shared")

    groups = [list(range(N_CORES))]
    Sig = mybir.ActivationFunctionType.Sigmoid
    Tanh = mybir.ActivationFunctionType.Tanh

    # Per-500-chunk activation map for the gate layout [i|f|g|o] * 625:
    # (chunk, lo, hi, func)
    act_map = [
        (0, 0, 500, Sig), (1, 0, 500, Sig),
        (2, 0, 250, Sig), (2, 250, 500, Tanh),
        (3, 0, 375, Tanh), (3, 375, 500, Sig),
        (4, 0, 500, Sig),
    ]

    with tile.TileContext(nc) as tc:
        with (
            tc.tile_pool(name="wpool", bufs=3) as wpool,
            tc.tile_pool(name="misc", bufs=1) as misc,
            tc.tile_pool(name="gates", bufs=1) as gpool,
            tc.tile_pool(name="tmps", bufs=1) as tpool,
            tc.tile_pool(name="psum", bufs=1, space="PSUM") as ppool,
        ):
            hwdge = [nc.sync, nc.scalar]
            dma_i = 0

            def wdma(dst, src):
                nonlocal dma_i
                hwdge[dma_i % 2].dma_start(out=dst, in_=src)
                dma_i += 1

            # --- small input DMAs (off the weight-stream critical path) ---
            # NB: single-partition DMA segments must stay <= 512B (128 f32)
            # or the NEFF fails to load under this runtime.
            vec1_sb = misc.tile([128, B1], DT, name="vec1sb")
            nc.gpsimd.dma_start(out=vec1_sb[:], in_=vec1_ext[:])
            vec2_sb = misc.tile([128, B2], DT, name="vec2sb")
            nc.gpsimd.dma_start(out=vec2_sb[:, BL:B2], in_=h2t_ext[:])
            c1_sb = misc.tile([1, HS], DT, name="c1sb")
            c2_sb = misc.tile([1, HS], DT, name="c2sb")
            for i in range(5):
                sl = slice(i * 125, (i + 1) * 125)
                nc.gpsimd.dma_start(out=c1_sb[:, sl], in_=c1s_ext[:, sl])
                nc.gpsimd.dma_start(out=c2_sb[:, sl], in_=c2s_ext[:, sl])

            def cell(w_ext, nblocks, vec_sb, c_sb, hpad_sb):
                """One LSTM cell: stream weights, matvec, activations -> h."""
                pg = [ppool.tile([1, 512], DT, name=f"pg{n}") for n in range(NCHUNK)]
                for b0 in range(0, nblocks, BPD):
                    nb = min(BPD, nblocks - b0)
                    wt = wpool.tile([128, nb, C], DT, tag="w")
                    wdma(wt[:],
                         w_ext[b0 * 128:(b0 + nb) * 128, :]
                         .rearrange("(n p) c -> p n c", p=128))
                    for j in range(nb):
                        b = b0 + j
                        for n in range(NCHUNK):
                            nc.tensor.matmul(
                                pg[n][:, 0:500],
                                vec_sb[:, b:b + 1],
                                wt[:, j, n * 500:(n + 1) * 500],
                                start=(b == 0), stop=(b == nblocks - 1),
                            )
                # activations: psum gates -> SBUF (sigmoid i,f,o / tanh g)
                gates = gpool.tile([1, C], DT, name="gates")
                for (ch, lo, hi, func) in act_map:
                    nc.scalar.activation(
                        gates[:, ch * 500 + lo: ch * 500 + hi],
                        pg[ch][:, lo:hi], func)
                i_ap = gates[:, 0:HS]
                f_ap = gates[:, HS:2 * HS]
                g_ap = gates[:, 2 * HS:3 * HS]
                o_ap = gates[:, 3 * HS:4 * HS]
                m1 = tpool.tile([1, HS], DT, name="m1")
                m2 = tpool.tile([1, HS], DT, name="m2")
                nc.vector.tensor_mul(m1[:], i_ap, g_ap)
                nc.vector.tensor_mul(m2[:], f_ap, c_sb[:])
                nc.vector.tensor_add(m2[:], m1[:], m2[:])      # c_new
                nc.scalar.activation(m1[:], m2[:], Tanh)        # tanh(c_new)
                nc.vector.tensor_mul(hpad_sb[:, 0:HS], o_ap, m1[:])

            # --- cell 1 ---
            h1pad = misc.tile([1, SEG], DT, name="h1pad")
            nc.vector.memset(h1pad[:], 0.0)
            nc.vector.memset(h1pad[:, HS:HS + 1], 1.0)
            cell(w1_ext, B1, vec1_sb, c1_sb, h1pad)
            for i in range(5):
                nc.gpsimd.dma_start(out=h1_bounce[i * 128:(i + 1) * 128],
                                    in_=h1pad[0:1, i * 128:(i + 1) * 128])
            nc.gpsimd.collective_compute(
                "AllGather", mybir.AluOpType.bypass, replica_groups=groups,
                ins=[h1_bounce.ap().opt()], outs=[h1_gath.ap().opt()])
            nc.gpsimd.dma_start(
                out=vec2_sb[:, 0:BL],
                in_=h1_gath.ap().rearrange("(b p) -> p b", p=128))

            # --- cell 2 ---
            h2pad = misc.tile([1, SEG], DT, name="h2pad")
            nc.vector.memset(h2pad[:], 0.0)
            nc.vector.memset(h2pad[:, HS:HS + 1], 1.0)
            cell(w2_ext, B2, vec2_sb, c2_sb, h2pad)
            for i in range(5):
                nc.gpsimd.dma_start(out=h2_bounce[i * 128:(i + 1) * 128],
                                    in_=h2pad[0:1, i * 128:(i + 1) * 128])
            nc.gpsimd.collective_compute(
                "AllGather", mybir.AluOpType.bypass, replica_groups=groups,
                ins=[h2_bounce.ap().opt()], outs=[h2_gath.ap().opt()])
            vecl_sb = misc.tile([128, BL], DT, name="veclsb")
            nc.gpsimd.dma_start(
                out=vecl_sb[:],
                in_=h2_gath.ap().rearrange("(b p) -> p b", p=128))

            # --- final linear (column-parallel, bias folded in) ---
            po = ppool.tile([1, 512], DT, name="po")
            for b0 in range(0, BL, BPD):
                nb = min(BPD, BL - b0)
                wt = wpool.tile([128, nb, OS], DT, tag="w")
                wdma(wt[:],
                     wl_ext[b0 * 128:(b0 + nb) * 128, :]
                     .rearrange("(n p) c -> p n c", p=128))
                for j in range(nb):
                    b = b0 + j
                    nc.tensor.matmul(
                        po[:, 0:OS], vecl_sb[:, b:b + 1], wt[:, j, :],
                        start=(b == 0), stop=(b == BL - 1))
            out_sb = misc.tile([1, OS], DT, name="outsb")
            nc.vector.tensor_copy(out_sb[:], po[:, 0:OS])
            for i in range(4):
                sl = slice(i * 125, (i + 1) * 125)
                nc.sync.dma_start(out=out_ext[:, sl], in_=out_sb[:, sl])

    nc.compile()
    return nc


def _gate_cols(w, r):
    """[in_dim, 2500] column block for core r: gate-major [i|f|g|o] x 625,
    transposed so rows are the contraction (input) dim."""
    ind = w.shape[1]
    outb = np.empty((ind, C), dtype=F32)
    for k in range(4):
        rows = slice(k * H_DIM + r * HS, k * H_DIM + (r + 1) * HS)
        outb[:, k * HS:(k + 1) * HS] = w[rows, :].T
    return outb


def _gate_bias(b_a, b_b, r):
    out = np.empty((C,), dtype=F32)
    for k in range(4):
        rows = slice(k * H_DIM + r * HS, k * H_DIM + (r + 1) * HS)
        out[k * HS:(k + 1) * HS] = b_a[rows] + b_b[rows]
    return out


def _prep_core(r, input_data, w_ih1, w_hh1, b_ih1, b_hh1,
               w_ih2, w_hh2, b_ih2, b_hh2, w_lin, b_lin,
               h_t, c_t, h2_t, c2_t):
    # --- W1: [x-seg 4096 | h-seg 5120] x 2500 ---
    w1 = np.zeros((R1, C), dtype=F32)
    w1[0:I_DIM] = _gate_cols(w_ih1, r)
    w1[I_DIM] = _gate_bias(b_ih1, b_hh1, r)
    w1[XSEG:XSEG + H_DIM] = _gate_cols(w_hh1, r)

    # --- W2: [gathered-h1 seg 5120 | h2_t seg 5120] x 2500 ---
    # gathered layout: rank q occupies [q*640, q*640+625) with h1 values,
    # slot q*640+625 holds 1.0 (bias hooked on rank 0's slot only).
    w2 = np.zeros((R2, C), dtype=F32)
    wih2c = _gate_cols(w_ih2, r)
    for q in range(N_CORES):
        w2[q * SEG:q * SEG + HS] = wih2c[q * HS:(q + 1) * HS]
    w2[HS] = _gate_bias(b_ih2, b_hh2, r)      # rank 0's 1.0 slot (row 625)
    w2[GATH:GATH + H_DIM] = _gate_cols(w_hh2, r)

    # --- W_lin: [gathered-h2 seg 5120] x 500, bias on rank0 1.0 slot ---
    wl = np.zeros((RL, OS), dtype=F32)
    wlT = w_lin[r * OS:(r + 1) * OS, :].T     # [5000, 500]
    for q in range(N_CORES):
        wl[q * SEG:q * SEG + HS] = wlT[q * HS:(q + 1) * HS]
    wl[HS] = b_lin[r * OS:(r + 1) * OS]

    vec1 = np.zeros((R1,), dtype=F32)
    vec1[0:I_DIM] = input_data[0]
    vec1[I_DIM] = 1.0
    vec1[XSEG:XSEG + H_DIM] = h_t[0]
    vec1 = np.ascontiguousarray(vec1.reshape(B1, 128).T)   # [128, B1]

    h2tv = np.zeros((HSEG,), dtype=F32)
    h2tv[0:H_DIM] = h2_t[0]
    h2tv = np.ascontiguousarray(h2tv.reshape(BL, 128).T)   # [128, BL]

    return {
        "w1": w1, "w2": w2, "wl": wl, "vec1": vec1, "h2t": h2tv,
        "c1s": np.ascontiguousarray(c_t[:, r * HS:(r + 1) * HS], dtype=F32),
        "c2s": np.ascontiguousarray(c2_t[:, r * HS:(r + 1) * HS], dtype=F32),
    }


def kernel(**inputs):
    global _CACHED_NC
    if _CACHED_NC is None:
        _CACHED_NC = _build_bass()
    nc = _CACHED_NC

    args = {k: np.asarray(v, dtype=F32) for k, v in inputs.items()}
    in_maps = [_prep_core(r, **args) for r in range(N_CORES)]

    res = run_bass_kernel_spmd(nc, in_maps, core_ids=list(range(N_CORES)))
    out = np.concatenate([res.results[r]["out"][0] for r in range(N_CORES)])
    return out.reshape(1, I_DIM).astype(np.float32)
